# revision 20
# baseline (speedup 1.0000x reference)
"""EGCL (E(n)-equivariant graph conv layer) Trainium2 Bass kernel.

Strategy (edge-parallel, sharded by destination node r):
  - Host: sort edges by r, split nodes into 8 contiguous ranges with ~equal
    edge counts. Each core owns all edges pointing INTO its node range, so
    every aggregation (m_i segment-sum, coord update) is core-local: no
    collectives at all.
  - Edges are packed into 512-edge "macro tiles" aligned to node boundaries,
    each covering <=64 node slots. One-hot segment matrices (S / S^T) are
    pure indexing data and are built on the host and streamed in.
  - Device pipeline per macro tile (transposed activations: features on
    partitions, edges on the free dim):
      z1^T = P1[r]-expand (S^T matmul) + We1_c^T @ h_c^T (transposed gather)
             + rad outer-product (via PE-transposed rad slabs + masked mms)
      a1^T = silu(z1^T + be1)         [ACT, bias fused]
      m^T  = silu(We2^T @ a1^T + be2)
      w    = coord-MLP from m^T, ending in natural-layout w via per-chunk mms
      m_ij natural via identity-matmul transposes -> segment-matmul agg
  - Node MLP over slot columns, residuals added in f32.
Host does indexing/permutation/casting only - all FLOPs are on device.
"""

import sys

sys.path.insert(0, "/opt/trn_rl_repo")

import numpy as np
import ml_dtypes

D = 128
EPT = 512  # edges per macro tile
SPT = 64  # node slots per macro tile
NCH = EPT // 128  # 128-edge chunks per macro tile
GM = 8  # macro tiles per dma_gather call (4096 idxs)
BM = 64  # macro tiles per compact gather table (<= 32768 edges)
NCORES = 8
EPS = 1e-8

BF16 = ml_dtypes.bfloat16


def _pack_core(nlo, nhi, counts, cum):
    """Greedy-pack nodes [nlo, nhi) into macro tiles of <=EPT edges and
    <=SPT consecutive nodes. Returns list of (node_start, node_end)."""
    tiles = []
    a = nlo
    while a < nhi:
        b = a
        edges = 0
        while b < nhi and (b - a) < SPT and edges + counts[b] <= EPT:
            edges += counts[b]
            b += 1
        assert b > a, f"node {a} has degree {counts[a]} > {EPT}"
        tiles.append((a, b))
        a = b
    return tiles


def _prep(h, coord, edge_index):
    """Host-side sharding / index prep. Pure permutation+casting, no math."""
    N = h.shape[0]
    E = edge_index.shape[1]
    r = edge_index[0].astype(np.int64)
    c = edge_index[1].astype(np.int64)

    perm = np.argsort(r, kind="stable")
    r_s = r[perm]
    c_s = c[perm]
    counts = np.bincount(r_s, minlength=N)
    cum = np.concatenate([[0], np.cumsum(counts)])  # cum[n] = first edge of n

    # split nodes into NCORES ranges with ~equal edge counts
    bounds = [0]
    for k in range(1, NCORES):
        t = E * k // NCORES
        n = int(np.searchsorted(cum, t, side="left"))
        bounds.append(min(max(n, bounds[-1]), N))
    bounds.append(N)

    core_tiles = [
        _pack_core(bounds[k], bounds[k + 1], counts, cum) for k in range(NCORES)
    ]
    T = max(len(t) for t in core_tiles)
    T = ((T + GM - 1) // GM) * GM  # multiple of GM so gather calls are uniform
    S = T * SPT
    NB = (T + BM - 1) // BM  # number of gather-table batches

    h_bf = h.astype(BF16)
    cores = []
    for k in range(NCORES):
        tiles = core_tiles[k]
        lid = np.full((T, EPT), 127, dtype=np.int64)  # local slot id, 127=pad
        cidx = np.zeros((T, EPT), dtype=np.int64)  # c node id (pad -> 0)
        slot_node = np.full(S, -1, dtype=np.int64)
        for t, (a, b) in enumerate(tiles):
            e0, e1 = cum[a], cum[b]
            ne = e1 - e0
            lid[t, :ne] = r_s[e0:e1] - a
            cidx[t, :ne] = c_s[e0:e1]
            slot_node[t * SPT : t * SPT + (b - a)] = np.arange(a, b)

        # one-hot segment matrices
        sl = np.arange(SPT)
        st = (lid[:, None, :] == sl[None, :, None]).astype(BF16)  # [T,64,512]
        lid_r = lid.reshape(T, NCH, 128)
        sn = (lid_r[:, :, :, None] == sl[None, None, None, :]).astype(
            BF16
        )  # [T,NCH,128,64]
        sn = sn.transpose(0, 2, 1, 3).reshape(T, 128, NCH * SPT)  # [T,128,NCH*64]

        # host-expanded h_c in transposed layout: col t*EPT+e = h_bf[c(t, e)]
        hct = np.ascontiguousarray(h_bf[cidx].reshape(T * EPT, D).T)  # [128, T*EPT]

        # host-expanded per-edge coords, layout [128, T*NCH, 3]
        rn = slot_node[(np.arange(T)[:, None] * SPT + lid).clip(0, S - 1)]
        rn = np.where(lid < SPT, rn, 0).clip(0, N - 1)  # pad -> node 0
        crd_r = coord[rn]  # [T,EPT,3]
        crd_r[lid >= SPT] = 0.0
        crd_c = coord[cidx]  # [T,EPT,3]
        cr_dev = crd_r.reshape(T * NCH, 128, 3).transpose(1, 0, 2)  # [128,T4,3]
        cc_dev = crd_c.reshape(T * NCH, 128, 3).transpose(1, 0, 2)

        sn_valid = slot_node >= 0
        hslot = np.zeros((S, D), dtype=np.float32)
        hslot[sn_valid] = h[slot_node[sn_valid]]
        cslot = np.zeros((S, 3), dtype=np.float32)
        cslot[sn_valid] = coord[slot_node[sn_valid]]

        cores.append(
            dict(
                lid=lid,
                cidx=cidx,
                slot_node=slot_node,
                st=np.ascontiguousarray(st),
                sn=np.ascontiguousarray(sn),
                hct=hct,
                crd_r=np.ascontiguousarray(cr_dev.astype(np.float32)),
                crd_c=np.ascontiguousarray(cc_dev.astype(np.float32)),
                hslotT=np.ascontiguousarray(hslot.T),  # [128,S] f32
                cslotT=np.ascontiguousarray(cslot.T),  # [3,S] f32
            )
        )

    return cores, T, S, NB, 0


def _weights_map(We1, be1, We2, be2, Wc1, bc1, Wc2, bc2, Wn1, bn1, Wn2, bn2):
    """Per-core replicated weight tensors (bf16 mats, f32 bias columns)."""
    col = lambda v: np.ascontiguousarray(
        np.broadcast_to(np.asarray(v, np.float32).reshape(-1, 1), (D, 1))
        if np.asarray(v).size in (1, D)
        else v
    )
    zcat = np.zeros((32, 32, D), dtype=BF16)
    for g in range(32):
        zcat[g, g] = We1[2 * D].astype(BF16)
    zcat = zcat.transpose(1, 0, 2).reshape(32, 32 * D)  # [k, g*D]
    return dict(
        we1r=We1[:D].astype(BF16),
        we1c=We1[D : 2 * D].astype(BF16),
        zcat=zcat,
        we2=We2.astype(BF16),
        wc1=Wc1.astype(BF16),
        wc2=Wc2.astype(BF16),
        wn1h=Wn1[:D].astype(BF16),
        wn1m=Wn1[D : 2 * D].astype(BF16),
        wn2=Wn2.astype(BF16),
        be1=col(be1),
        be2=col(be2),
        bc1=col(bc1),
        bc2=col(np.full(D, float(np.asarray(bc2).reshape(-1)[0]), np.float32)),
        bn1=col(bn1),
        bn2=col(bn2),
    )


def build_program(T, S, NB, TBLR):
    import concourse.bass as bass
    import concourse.mybir as mybir
    import concourse.tile as tile
    from concourse import bacc
    from concourse.masks import make_identity

    f32 = mybir.dt.float32
    bf16 = mybir.dt.bfloat16
    AF = mybir.ActivationFunctionType
    T4 = T * NCH
    NSLAB = (T4 + 31) // 32

    nc = bacc.Bacc("TRN2", target_bir_lowering=False, debug=False)
    g = lambda n, s, d: nc.declare_dram_parameter(n, list(s), d, isOutput=False)
    hct_d = g("hct", (128, T * EPT), bf16)
    st_d = g("st", (T, SPT, EPT), bf16)
    sn_d = g("sn", (T, 128, NCH * SPT), bf16)
    crd_r_d = g("crd_r", (128, T4 * 3), f32)
    crd_c_d = g("crd_c", (128, T4 * 3), f32)
    hslotT_d = g("hslotT", (128, S), f32)
    cslotT_d = g("cslotT", (3, S), f32)
    wnames = [
        ("we1r", (D, D), bf16),
        ("we1c", (D, D), bf16),
        ("zcat", (32, 32 * D), bf16),
        ("we2", (D, D), bf16),
        ("wc1", (D, D), bf16),
        ("wc2", (D, 1), bf16),
        ("wn1h", (D, D), bf16),
        ("wn1m", (D, D), bf16),
        ("wn2", (D, D), bf16),
        ("be1", (D, 1), f32),
        ("be2", (D, 1), f32),
        ("bc1", (D, 1), f32),
        ("bc2", (D, 1), f32),
        ("bn1", (D, 1), f32),
        ("bn2", (D, 1), f32),
    ]
    wd = {n: g(n, s, d) for n, s, d in wnames}
    houtT_d = nc.declare_dram_parameter("houtT", [128, S], f32, isOutput=True)
    coutT_d = nc.declare_dram_parameter("coutT", [3, S], f32, isOutput=True)

    with tile.TileContext(nc) as tc:
        with (
            tc.tile_pool(name="res", bufs=1) as res,
            tc.tile_pool(name="work", bufs=2) as work,
            tc.tile_pool(name="gath", bufs=2) as gath,
            tc.tile_pool(name="act", bufs=2) as actp,
            tc.tile_pool(name="ps_big", bufs=4, space="PSUM") as ps_big,
        ):
            # ---- phase 0: residents ----
            w_sb = {}
            for n, s, d in wnames:
                w_sb[n] = res.tile(list(s), d, name=f"w_{n}")
                nc.sync.dma_start(out=w_sb[n][:], in_=wd[n][:])
            ident = res.tile([128, 128], bf16, name="ident")
            make_identity(nc, ident[:])
            identf = res.tile([128, 128], f32, name="identf")
            make_identity(nc, identf[:])

            hslotb = res.tile([128, S], bf16, name="hslotb")
            for k in range(S // EPT):
                hs_t = work.tile([128, EPT], f32, tag="hs", name="hs_t")
                nc.sync.dma_start(
                    out=hs_t[:], in_=hslotT_d[:, k * EPT : (k + 1) * EPT]
                )
                nc.vector.tensor_copy(
                    out=hslotb[:, k * EPT : (k + 1) * EPT], in_=hs_t[:]
                )
            mi_sb = res.tile([128, S], bf16, name="mi_sb")
            diffn = res.tile([128, T4, 3], bf16, name="diffn")
            radt = res.tile([32, NSLAB * 128], bf16, name="radt")
            p1 = res.tile([128, (T + 1) // 2, D], bf16, name="p1")

            # ---- P1 = h_slot @ We1_r (two macro tiles per 128-slot chunk) ----
            ps_init_cm = tc.tile_pool(name="ps_init", bufs=2, space="PSUM")
            ps_init = ps_init_cm.__enter__()
            for k in range(S // 128):
                pk = ps_init.tile([128, 128], f32, tag="p1ps", name="pk")
                nc.tensor.matmul(
                    out=pk[:],
                    lhsT=hslotb[:, k * 128 : (k + 1) * 128],
                    rhs=w_sb["we1r"][:],
                    start=True,
                    stop=True,
                )
                nc.vector.tensor_copy(out=p1[:, k, :], in_=pk[:])

            # ---- phase A: coord diff / rad / normalized diff ----
            crd_r = work.tile([128, T4, 3], f32, tag="crd", name="crd_r")
            crd_c = work.tile([128, T4, 3], f32, tag="crd", name="crd_c")
            nc.sync.dma_start(out=crd_r[:], in_=crd_r_d[:])
            nc.sync.dma_start(out=crd_c[:], in_=crd_c_d[:])
            rad = work.tile([128, T4], f32, tag="rad", name="rad")
            diff = work.tile([128, T4, 3], f32, tag="diff", name="diff")
            nc.vector.tensor_tensor(
                out=diff[:], in0=crd_r[:], in1=crd_c[:], op=mybir.AluOpType.subtract
            )
            sq = work.tile([128, T4, 3], f32, tag="crd", name="sq")
            nc.vector.tensor_tensor(
                out=sq[:], in0=diff[:], in1=diff[:], op=mybir.AluOpType.mult
            )
            nc.vector.reduce_sum(out=rad[:], in_=sq[:], axis=mybir.AxisListType.X)
            rnorm = work.tile([128, T4], f32, tag="rn", name="rnorm")
            nc.scalar.activation(out=rnorm[:], in_=rad[:], func=AF.Sqrt)
            nc.vector.tensor_scalar_add(out=rnorm[:], in0=rnorm[:], scalar1=EPS)
            nc.vector.reciprocal(out=rnorm[:], in_=rnorm[:])
            nc.vector.tensor_tensor(
                out=diffn[:],
                in0=diff[:],
                in1=rnorm[:].to_broadcast([128, T4, 3]),
                op=mybir.AluOpType.mult,
            )
            # rad slabs: radt[k, s*128 + m] = rad[m, 32s + k]
            for s in range(NSLAB):
                w32 = min(32, T4 - s * 32)
                rtp = ps_init.tile([32, 128], f32, tag="rtp", name="rtp")
                nc.tensor.transpose(
                    out=rtp[:w32, :],
                    in_=rad[:, s * 32 : s * 32 + w32],
                    identity=identf[:],
                )
                nc.vector.tensor_copy(
                    out=radt[:w32, s * 128 : (s + 1) * 128], in_=rtp[:w32, :]
                )

            ps_init_cm.__exit__(None, None, None)
            ps_sm_cm = tc.tile_pool(name="ps_sm", bufs=2, space="PSUM")
            ps_sm = ps_sm_cm.__enter__()

            # ---- phase B: per macro tile ----
            gbuf = None
            for t in range(T):
                if t % GM == 0:
                    gi = t // GM
                    gbuf = gath.tile([128, GM * EPT], bf16, tag="gb", name="gbuf")
                    nc.sync.dma_start(
                        out=gbuf[:],
                        in_=hct_d[:, gi * GM * EPT : (gi + 1) * GM * EPT],
                    )
                pb = SPT * (t % 2)
                st_t = work.tile([128, EPT], bf16, tag="st", name="st_t")
                nc.sync.dma_start(out=st_t[pb : pb + SPT, :], in_=st_d[t])
                sn_t = work.tile([128, NCH * SPT], bf16, tag="sn", name="sn_t")
                nc.sync.dma_start(out=sn_t[:], in_=sn_d[t])

                # z1^T accumulation
                z1 = ps_big.tile([128, EPT], f32, tag="big", name="z1")
                nc.tensor.matmul(
                    out=z1[:],
                    lhsT=p1[pb : pb + SPT, t // 2, :],
                    rhs=st_t[pb : pb + SPT, :],
                    start=True,
                    stop=False,
                )
                for j in range(NCH):
                    gch = t * NCH + j
                    nc.tensor.matmul(
                        out=z1[:, j * 128 : (j + 1) * 128],
                        lhsT=w_sb["zcat"][:, (gch % 32) * D : (gch % 32 + 1) * D],
                        rhs=radt[:, (gch // 32) * 128 : (gch // 32 + 1) * 128],
                        start=False,
                        stop=False,
                        skip_group_check=True,
                    )
                off = (t % GM) * EPT
                nc.tensor.matmul(
                    out=z1[:],
                    lhsT=w_sb["we1c"][:],
                    rhs=gbuf[:, off : off + EPT],
                    start=False,
                    stop=True,
                )
                a1 = actp.tile([128, EPT], bf16, tag="a1", name="a1")
                nc.scalar.activation(
                    out=a1[:], in_=z1[:], func=AF.Silu, bias=w_sb["be1"][:]
                )
                z2 = ps_big.tile([128, EPT], f32, tag="big", name="z2")
                nc.tensor.matmul(out=z2[:], lhsT=w_sb["we2"][:], rhs=a1[:])
                mT = actp.tile([128, EPT], bf16, tag="mT", name="mT")
                nc.scalar.activation(
                    out=mT[:], in_=z2[:], func=AF.Silu, bias=w_sb["be2"][:]
                )
                c1 = ps_big.tile([128, EPT], f32, tag="big", name="c1")
                nc.tensor.matmul(out=c1[:], lhsT=w_sb["wc1"][:], rhs=mT[:])
                ac1 = actp.tile([128, EPT], bf16, tag="ac1", name="ac1")
                nc.scalar.activation(
                    out=ac1[:], in_=c1[:], func=AF.Silu, bias=w_sb["bc1"][:]
                )
                # w natural: per-chunk (ac1 chunk)^T @ Wc2 -> [128e, 1]
                cw = ps_sm.tile([128, SPT + NCH], f32, tag="cw", name="cw")
                for j in range(NCH):
                    nc.tensor.matmul(
                        out=cw[:, SPT + j : SPT + j + 1],
                        lhsT=ac1[:, j * 128 : (j + 1) * 128],
                        rhs=w_sb["wc2"][:],
                        start=True,
                        stop=True,
                        skip_group_check=True,
                    )
                # m_ij natural via identity matmuls
                mn_ps = ps_big.tile([128, EPT], f32, tag="big", name="mn_ps")
                for j in range(NCH):
                    nc.tensor.matmul(
                        out=mn_ps[:, j * 128 : (j + 1) * 128],
                        lhsT=mT[:, j * 128 : (j + 1) * 128],
                        rhs=ident[:],
                        start=(j == 0),
                        stop=(j == NCH - 1),
                        skip_group_check=True,
                    )
                mn = actp.tile([128, EPT], bf16, tag="mn", name="mn")
                nc.vector.tensor_copy(out=mn[:], in_=mn_ps[:])
                wcol = work.tile([128, NCH], f32, tag="wcol", name="wcol")
                nc.vector.tensor_scalar(
                    out=wcol[:],
                    in0=cw[:, SPT : SPT + NCH],
                    scalar1=w_sb["bc2"][:],
                    scalar2=None,
                    op0=mybir.AluOpType.add,
                )
                dnw = work.tile([128, NCH, 3], bf16, tag="dnw", name="dnw")
                nc.vector.tensor_tensor(
                    out=dnw[:],
                    in0=diffn[:, t * NCH : (t + 1) * NCH, :],
                    in1=wcol[:].to_broadcast([128, NCH, 3]),
                    op=mybir.AluOpType.mult,
                )
                mi_ps = ps_sm.tile([128, SPT], f32, tag="mi", name="mi_ps")
                for j in range(NCH):
                    nc.tensor.matmul(
                        out=mi_ps[:],
                        lhsT=mn[:, j * 128 : (j + 1) * 128],
                        rhs=sn_t[:, j * SPT : (j + 1) * SPT],
                        start=(j == 0),
                        stop=(j == NCH - 1),
                    )
                for j in range(NCH):
                    nc.tensor.matmul(
                        out=cw[0:3, 0:SPT],
                        lhsT=dnw[:, j, :],
                        rhs=sn_t[:, j * SPT : (j + 1) * SPT],
                        start=(j == 0),
                        stop=(j == NCH - 1),
                        skip_group_check=True,
                    )
                nc.vector.tensor_copy(
                    out=mi_sb[:, t * SPT : (t + 1) * SPT], in_=mi_ps[:]
                )
                if t % GM == 0:
                    cagg8 = work.tile([3, GM * SPT], f32, tag="cagg8", name="cagg8")
                nc.vector.tensor_copy(
                    out=cagg8[:, (t % GM) * SPT : (t % GM + 1) * SPT],
                    in_=cw[0:3, 0:SPT],
                )
                if t % GM == GM - 1:
                    csl = work.tile([3, GM * SPT], f32, tag="csl", name="csl")
                    g0 = (t // GM) * GM * SPT
                    nc.sync.dma_start(
                        out=csl[:], in_=cslotT_d[:, g0 : g0 + GM * SPT]
                    )
                    nc.vector.tensor_tensor(
                        out=csl[:], in0=csl[:], in1=cagg8[:],
                        op=mybir.AluOpType.add,
                    )
                    nc.sync.dma_start(
                        out=coutT_d[:, g0 : g0 + GM * SPT], in_=csl[:]
                    )

            # ---- phase C: node MLP + residuals ----
            for k in range(S // EPT):
                sl = slice(k * EPT, (k + 1) * EPT)
                zn = ps_big.tile([128, EPT], f32, tag="big", name="zn")
                nc.tensor.matmul(
                    out=zn[:], lhsT=w_sb["wn1h"][:], rhs=hslotb[:, sl],
                    start=True, stop=False,
                )
                nc.tensor.matmul(
                    out=zn[:], lhsT=w_sb["wn1m"][:], rhs=mi_sb[:, sl],
                    start=False, stop=True,
                )
                an = actp.tile([128, EPT], bf16, tag="a1", name="an")
                nc.scalar.activation(
                    out=an[:], in_=zn[:], func=AF.Silu, bias=w_sb["bn1"][:]
                )
                zn2 = ps_big.tile([128, EPT], f32, tag="big", name="zn2")
                nc.tensor.matmul(out=zn2[:], lhsT=w_sb["wn2"][:], rhs=an[:])
                ho = work.tile([128, EPT], f32, tag="ho", name="ho")
                hres = work.tile([128, EPT], f32, tag="hs", name="hres")
                nc.sync.dma_start(out=hres[:], in_=hslotT_d[:, sl])
                nc.vector.tensor_scalar(
                    out=ho[:],
                    in0=zn2[:],
                    scalar1=w_sb["bn2"][:],
                    scalar2=None,
                    op0=mybir.AluOpType.add,
                )
                nc.vector.tensor_tensor(
                    out=ho[:], in0=ho[:], in1=hres[:], op=mybir.AluOpType.add
                )
                nc.sync.dma_start(out=houtT_d[:, sl], in_=ho[:])
            ps_sm_cm.__exit__(None, None, None)

    nc.compile()
    return nc


def kernel(h, coord, edge_index, We1, be1, We2, be2, Wc1, bc1, Wc2, bc2,
           Wn1, bn1, Wn2, bn2, _run=None):
    h = np.asarray(h, np.float32)
    coord = np.asarray(coord, np.float32)
    edge_index = np.asarray(edge_index)
    cores, T, S, NB, TBLR = _prep(h, coord, edge_index)
    wmap = _weights_map(We1, be1, We2, be2, Wc1, bc1, Wc2, bc2, Wn1, bn1, Wn2, bn2)

    nc = build_program(T, S, NB, TBLR)

    in_maps = []
    for cd in cores:
        m = dict(
            hct=cd["hct"],
            st=np.asarray(cd["st"]).reshape(T, SPT, EPT),
            sn=np.asarray(cd["sn"]),
            crd_r=cd["crd_r"].reshape(128, -1),
            crd_c=cd["crd_c"].reshape(128, -1),
            hslotT=cd["hslotT"],
            cslotT=cd["cslotT"],
        )
        for k, v in wmap.items():
            m[k] = v
        in_maps.append(m)

    if _run is None:
        from concourse.bass_utils import run_bass_kernel_spmd

        res = run_bass_kernel_spmd(nc, in_maps, list(range(NCORES)))
        outs = res.results
    else:
        outs = _run(nc, in_maps)

    N = h.shape[0]
    h_out = np.zeros((N, D), dtype=np.float32)
    coord_out = np.zeros((N, 3), dtype=np.float32)
    for cd, om in zip(cores, outs):
        sn_ = cd["slot_node"]
        v = sn_ >= 0
        h_out[sn_[v]] = np.asarray(om["houtT"]).T[v]
        coord_out[sn_[v]] = np.asarray(om["coutT"]).T[v]
    return h_out, coord_out


# revision 27
# speedup vs baseline: 1.7945x; 1.7945x over previous
"""EGCL (E(n)-equivariant graph conv layer) Trainium2 Bass kernel.

Strategy (edge-parallel, sharded by destination node r):
  - Host: sort edges by r, split nodes into 8 contiguous ranges with ~equal
    edge counts. Each core owns all edges pointing INTO its node range, so
    every aggregation (m_i segment-sum, coord update) is core-local: no
    collectives at all.
  - Edges are packed into 512-edge "macro tiles" aligned to node boundaries,
    each covering <=64 node slots. One-hot segment matrices (S / S^T) are
    pure indexing data and are built on the host and streamed in.
  - Device pipeline per macro tile (transposed activations: features on
    partitions, edges on the free dim):
      z1^T = P1[r]-expand (S^T matmul) + We1_c^T @ h_c^T (transposed gather)
             + rad outer-product (via PE-transposed rad slabs + masked mms)
      a1^T = silu(z1^T + be1)         [ACT, bias fused]
      m^T  = silu(We2^T @ a1^T + be2)
      w    = coord-MLP from m^T, ending in natural-layout w via per-chunk mms
      m_ij natural via identity-matmul transposes -> segment-matmul agg
  - Node MLP over slot columns, residuals added in f32.
Host does indexing/permutation/casting only - all FLOPs are on device.
"""

import sys

sys.path.insert(0, "/opt/trn_rl_repo")

import numpy as np
import ml_dtypes

D = 128
EPT = 512  # edges per macro tile
SPT = 64  # node slots per macro tile
NCH = EPT // 128  # 128-edge chunks per macro tile
GMS = 4  # macro tiles per packed stream group
NCORES = 8
EPS = 1e-8

BF16 = ml_dtypes.bfloat16


def _pack_core(nlo, nhi, counts, cum):
    """Greedy-pack nodes [nlo, nhi) into macro tiles of <=EPT edges and
    <=SPT consecutive nodes. Returns list of (node_start, node_end)."""
    tiles = []
    a = nlo
    while a < nhi:
        b = a
        edges = 0
        while b < nhi and (b - a) < SPT and edges + counts[b] <= EPT:
            edges += counts[b]
            b += 1
        assert b > a, f"node {a} has degree {counts[a]} > {EPT}"
        tiles.append((a, b))
        a = b
    return tiles


def _prep(h, coord, edge_index):
    """Host-side sharding / index prep. Pure permutation+casting, no math."""
    N = h.shape[0]
    E = edge_index.shape[1]
    r = edge_index[0].astype(np.int64)
    c = edge_index[1].astype(np.int64)

    perm = np.argsort(r, kind="stable")
    r_s = r[perm]
    c_s = c[perm]
    counts = np.bincount(r_s, minlength=N)
    cum = np.concatenate([[0], np.cumsum(counts)])  # cum[n] = first edge of n

    # split nodes into NCORES ranges with ~equal edge counts
    bounds = [0]
    for k in range(1, NCORES):
        t = E * k // NCORES
        n = int(np.searchsorted(cum, t, side="left"))
        bounds.append(min(max(n, bounds[-1]), N))
    bounds.append(N)

    core_tiles = [
        _pack_core(bounds[k], bounds[k + 1], counts, cum) for k in range(NCORES)
    ]
    T = max(len(t) for t in core_tiles)
    T = ((T + 7) // 8) * 8  # multiple of 8: stream groups and EPT-wide slots
    S = T * SPT
    NB = 0

    h_bf = h.astype(BF16)
    cores = []
    for k in range(NCORES):
        tiles = core_tiles[k]
        lid = np.full((T, EPT), 127, dtype=np.int64)  # local slot id, 127=pad
        cidx = np.zeros((T, EPT), dtype=np.int64)  # c node id (pad -> 0)
        slot_node = np.full(S, -1, dtype=np.int64)
        for t, (a, b) in enumerate(tiles):
            e0, e1 = cum[a], cum[b]
            ne = e1 - e0
            lid[t, :ne] = r_s[e0:e1] - a
            cidx[t, :ne] = c_s[e0:e1]
            slot_node[t * SPT : t * SPT + (b - a)] = np.arange(a, b)

        # one-hot segment matrices
        sl = np.arange(SPT)
        st = (lid[:, None, :] == sl[None, :, None]).astype(BF16)  # [T,64,512]
        lid_r = lid.reshape(T, NCH, 128)
        sn = (lid_r[:, :, :, None] == sl[None, None, None, :]).astype(
            BF16
        )  # [T,NCH,128,64]
        sn = sn.transpose(0, 2, 1, 3).reshape(T, 128, NCH * SPT)  # [T,128,NCH*64]

        # host-expanded h_c in transposed layout: col t*EPT+e = h_bf[c(t, e)]
        hct = h_bf[cidx].reshape(T * EPT, D).T  # [128, T*EPT]
        # packed per-group stream: hcT + ST (parity-placed); SN separate
        G = T // GMS
        grp = np.zeros((G, 128, GMS, 2 * EPT), dtype=BF16)
        snb = np.zeros((G, 128, GMS, NCH * SPT), dtype=BF16)
        hct3 = hct.reshape(128, T, EPT)
        for t in range(T):
            gg, m = divmod(t, GMS)
            grp[gg, :, m, 0:EPT] = hct3[:, t, :]
            pb = SPT * (t % 2)
            grp[gg, pb : pb + SPT, m, EPT : 2 * EPT] = st[t]
            snb[gg, :, m, :] = sn[t]

        # host-expanded per-edge coords, layout [128, T*NCH, 3]
        rn = slot_node[(np.arange(T)[:, None] * SPT + lid).clip(0, S - 1)]
        rn = np.where(lid < SPT, rn, 0).clip(0, N - 1)  # pad -> node 0
        crd_r = coord[rn]  # [T,EPT,3]
        crd_r[lid >= SPT] = 0.0
        crd_c = coord[cidx]  # [T,EPT,3]
        cr_dev = crd_r.reshape(T * NCH, 128, 3).transpose(1, 0, 2)  # [128,T4,3]
        cc_dev = crd_c.reshape(T * NCH, 128, 3).transpose(1, 0, 2)

        sn_valid = slot_node >= 0
        hslot = np.zeros((S, D), dtype=np.float32)
        hslot[sn_valid] = h[slot_node[sn_valid]]
        cslot = np.zeros((S, 3), dtype=np.float32)
        cslot[sn_valid] = coord[slot_node[sn_valid]]

        cores.append(
            dict(
                lid=lid,
                cidx=cidx,
                slot_node=slot_node,
                grp=np.ascontiguousarray(grp),
                snb=np.ascontiguousarray(snb),
                crd_r=np.ascontiguousarray(cr_dev.astype(np.float32)),
                crd_c=np.ascontiguousarray(cc_dev.astype(np.float32)),
                hslotT=np.ascontiguousarray(hslot.T),  # [128,S] f32
                cslotT=np.ascontiguousarray(cslot.T),  # [3,S] f32
            )
        )

    return cores, T, S, NB, 0


def _weights_map(We1, be1, We2, be2, Wc1, bc1, Wc2, bc2, Wn1, bn1, Wn2, bn2):
    """Per-core replicated weight tensors (bf16 mats, f32 bias columns)."""
    col = lambda v: np.ascontiguousarray(
        np.broadcast_to(np.asarray(v, np.float32).reshape(-1, 1), (D, 1))
        if np.asarray(v).size in (1, D)
        else v
    )
    zcat = np.zeros((32, 32, D), dtype=BF16)
    for g in range(32):
        zcat[g, g] = We1[2 * D].astype(BF16)
    zcat = zcat.transpose(1, 0, 2).reshape(32, 32 * D)  # [k, g*D]
    return dict(
        we1r=We1[:D].astype(BF16),
        we1c=We1[D : 2 * D].astype(BF16),
        zcat=zcat,
        we2=We2.astype(BF16),
        wc1=Wc1.astype(BF16),
        wc2=Wc2.astype(BF16),
        wn1h=Wn1[:D].astype(BF16),
        wn1m=Wn1[D : 2 * D].astype(BF16),
        wn2=Wn2.astype(BF16),
        be1=col(be1),
        be2=col(be2),
        bc1=col(bc1),
        bc2=col(np.full(D, float(np.asarray(bc2).reshape(-1)[0]), np.float32)),
        bn1=col(bn1),
        bn2=col(bn2),
    )


def build_program(T, S, NB, TBLR):
    import concourse.bass as bass
    import concourse.mybir as mybir
    import concourse.tile as tile
    from concourse import bacc
    from concourse.masks import make_identity

    f32 = mybir.dt.float32
    bf16 = mybir.dt.bfloat16
    AF = mybir.ActivationFunctionType
    T4 = T * NCH
    NSLAB = (T4 + 31) // 32

    nc = bacc.Bacc("TRN2", target_bir_lowering=False, debug=False)
    g = lambda n, s, d: nc.declare_dram_parameter(n, list(s), d, isOutput=False)
    G = T // GMS
    grp_d = g("grp", (G, 128, GMS * 2 * EPT), bf16)
    snb_d = g("snb", (G, 128, GMS * NCH * SPT), bf16)
    crd_r_d = g("crd_r", (128, T4 * 3), f32)
    crd_c_d = g("crd_c", (128, T4 * 3), f32)
    hslotT_d = g("hslotT", (128, S), f32)
    cslotT_d = g("cslotT", (3, S), f32)
    wnames = [
        ("we1r", (D, D), bf16),
        ("we1c", (D, D), bf16),
        ("zcat", (32, 32 * D), bf16),
        ("we2", (D, D), bf16),
        ("wc1", (D, D), bf16),
        ("wc2", (D, 1), bf16),
        ("wn1h", (D, D), bf16),
        ("wn1m", (D, D), bf16),
        ("wn2", (D, D), bf16),
        ("be1", (D, 1), f32),
        ("be2", (D, 1), f32),
        ("bc1", (D, 1), f32),
        ("bc2", (D, 1), f32),
        ("bn1", (D, 1), f32),
        ("bn2", (D, 1), f32),
    ]
    wd = {n: g(n, s, d) for n, s, d in wnames}
    houtT_d = nc.declare_dram_parameter("houtT", [128, S], f32, isOutput=True)
    coutT_d = nc.declare_dram_parameter("coutT", [3, S], f32, isOutput=True)

    with tile.TileContext(nc) as tc:
        with (
            tc.tile_pool(name="res", bufs=1) as res,
            tc.tile_pool(name="work", bufs=2) as work,
            tc.tile_pool(name="gath", bufs=3) as gath,
            tc.tile_pool(name="act", bufs=2) as actp,
            tc.tile_pool(name="ps_big", bufs=6, space="PSUM") as ps_big,
        ):
            # ---- phase 0: residents ----
            w_sb = {}
            for n, s, d in wnames:
                w_sb[n] = res.tile(list(s), d, name=f"w_{n}")
                nc.sync.dma_start(out=w_sb[n][:], in_=wd[n][:])
            ident = res.tile([128, 128], bf16, name="ident")
            make_identity(nc, ident[:])
            identf = res.tile([128, 128], f32, name="identf")
            make_identity(nc, identf[:])

            hslotb = res.tile([128, S], bf16, name="hslotb")
            for k in range(S // EPT):
                hs_t = work.tile([128, EPT], f32, tag="hs", name="hs_t")
                nc.sync.dma_start(
                    out=hs_t[:], in_=hslotT_d[:, k * EPT : (k + 1) * EPT]
                )
                nc.vector.tensor_copy(
                    out=hslotb[:, k * EPT : (k + 1) * EPT], in_=hs_t[:]
                )
            mi_sb = res.tile([128, S], bf16, name="mi_sb")
            diffn = res.tile([128, T4, 3], bf16, name="diffn")
            radt = res.tile([32, NSLAB * 128], bf16, name="radt")
            p1 = res.tile([128, (T + 1) // 2, D], bf16, name="p1")

            # ---- P1 = h_slot @ We1_r (two macro tiles per 128-slot chunk) ----
            for k in range(S // 128):
                pk = ps_big.tile([128, 512], f32, tag="big", name="pk")
                nc.tensor.matmul(
                    out=pk[:, 0:128],
                    lhsT=hslotb[:, k * 128 : (k + 1) * 128],
                    rhs=w_sb["we1r"][:],
                    start=True,
                    stop=True,
                )
                nc.vector.tensor_copy(out=p1[:, k, :], in_=pk[:, 0:128])

            # ---- phase A: coord diff / rad / normalized diff ----
            crd_r = work.tile([128, T4, 3], f32, tag="crd", name="crd_r")
            crd_c = work.tile([128, T4, 3], f32, tag="crd", name="crd_c")
            nc.sync.dma_start(out=crd_r[:], in_=crd_r_d[:])
            nc.sync.dma_start(out=crd_c[:], in_=crd_c_d[:])
            rad = work.tile([128, T4], f32, tag="rad", name="rad")
            diff = work.tile([128, T4, 3], f32, tag="diff", name="diff")
            nc.vector.tensor_tensor(
                out=diff[:], in0=crd_r[:], in1=crd_c[:], op=mybir.AluOpType.subtract
            )
            sq = work.tile([128, T4, 3], f32, tag="crd", name="sq")
            nc.vector.tensor_tensor(
                out=sq[:], in0=diff[:], in1=diff[:], op=mybir.AluOpType.mult
            )
            nc.vector.reduce_sum(out=rad[:], in_=sq[:], axis=mybir.AxisListType.X)
            rnorm = work.tile([128, T4], f32, tag="rn", name="rnorm")
            nc.scalar.activation(out=rnorm[:], in_=rad[:], func=AF.Sqrt)
            nc.vector.tensor_scalar_add(out=rnorm[:], in0=rnorm[:], scalar1=EPS)
            nc.vector.reciprocal(out=rnorm[:], in_=rnorm[:])
            nc.vector.tensor_tensor(
                out=diffn[:],
                in0=diff[:],
                in1=rnorm[:].to_broadcast([128, T4, 3]),
                op=mybir.AluOpType.mult,
            )
            # rad slabs: radt[k, s*128 + m] = rad[m, 32s + k]
            for s in range(NSLAB):
                w32 = min(32, T4 - s * 32)
                rtp = ps_big.tile([32, 512], f32, tag="big", name="rtp")
                nc.tensor.transpose(
                    out=rtp[:w32, 0:128],
                    in_=rad[:, s * 32 : s * 32 + w32],
                    identity=identf[:],
                )
                nc.vector.tensor_copy(
                    out=radt[:w32, s * 128 : (s + 1) * 128], in_=rtp[:w32, 0:128]
                )

            # ---- phase B: software-pipelined macro tiles ----
            # stage offsets (iteration u runs t at lag 0):
            #   u:   z1-MMs(t)
            #   u+1: a1-silu(t)
            #   u+2: z2-MM(t), mT-silu(t)
            #   u+3: c1-MM(t), ac1-silu(t)
            #   u+4: mn-MMs(t), mn-cast(t)
            #   u+5: w-MMs(t), wcol/dnw(t), mi-MMs(t), cagg-MMs(t), copies(t)
            LAG = 5
            GRP, SNB, Z1, A1, Z2, MT, C1, AC1, MNP, MN, DNW = ({} for _ in range(11))
            CAGG8 = {}

            def load_group(gg):
                if gg >= G:
                    return
                gt = gath.tile([128, GMS, 2 * EPT], bf16, tag="grp", name="gt")
                nc.sync.dma_start(out=gt[:], in_=grp_d[gg])
                GRP[gg] = gt
                sb = gath.tile(
                    [128, GMS, NCH * SPT], bf16, tag="snb", bufs=4, name="sb"
                )
                nc.sync.dma_start(out=sb[:], in_=snb_d[gg])
                SNB[gg] = sb

            load_group(0)
            for u in range(T + LAG):
                t = u
                if t < T:
                    if t % GMS == 0:
                        load_group(t // GMS + 1)
                    gt = GRP[t // GMS]
                    m = t % GMS
                    pb = SPT * (t % 2)
                    z1 = ps_big.tile([128, EPT], f32, tag="big", name="z1")
                    nc.tensor.matmul(
                        out=z1[:],
                        lhsT=p1[pb : pb + SPT, t // 2, :],
                        rhs=gt[pb : pb + SPT, m, EPT : 2 * EPT],
                        start=True,
                        stop=False,
                    )
                    for j in range(NCH):
                        gch = t * NCH + j
                        nc.tensor.matmul(
                            out=z1[:, j * 128 : (j + 1) * 128],
                            lhsT=w_sb["zcat"][:, (gch % 32) * D : (gch % 32 + 1) * D],
                            rhs=radt[:, (gch // 32) * 128 : (gch // 32 + 1) * 128],
                            start=False,
                            stop=False,
                            skip_group_check=True,
                        )
                    nc.tensor.matmul(
                        out=z1[:],
                        lhsT=w_sb["we1c"][:],
                        rhs=gt[:, m, 0:EPT],
                        start=False,
                        stop=True,
                    )
                    Z1[t] = z1
                t = u - 1
                if 0 <= t < T:
                    a1 = actp.tile([128, EPT], bf16, tag="a1", bufs=3, name="a1")
                    nc.scalar.activation(
                        out=a1[:], in_=Z1.pop(t)[:], func=AF.Silu, bias=w_sb["be1"][:]
                    )
                    A1[t] = a1
                t = u - 2
                if 0 <= t < T:
                    z2 = ps_big.tile([128, EPT], f32, tag="big", name="z2")
                    nc.tensor.matmul(out=z2[:], lhsT=w_sb["we2"][:], rhs=A1.pop(t)[:])
                    mT = actp.tile([128, EPT], bf16, tag="mT", bufs=4, name="mT")
                    nc.scalar.activation(
                        out=mT[:], in_=z2[:], func=AF.Silu, bias=w_sb["be2"][:]
                    )
                    Z2[t] = z2
                    MT[t] = mT
                t = u - 3
                if 0 <= t < T:
                    Z2.pop(t)
                    c1 = ps_big.tile([128, EPT], f32, tag="big", name="c1")
                    nc.tensor.matmul(out=c1[:], lhsT=w_sb["wc1"][:], rhs=MT[t][:])
                    ac1 = actp.tile([128, EPT], bf16, tag="ac1", bufs=4, name="ac1")
                    nc.scalar.activation(
                        out=ac1[:], in_=c1[:], func=AF.Silu, bias=w_sb["bc1"][:]
                    )
                    C1[t] = c1
                    AC1[t] = ac1
                t = u - 4
                if 0 <= t < T:
                    C1.pop(t)
                    mT = MT.pop(t)
                    mnp = ps_big.tile([128, EPT], f32, tag="mn", bufs=1, name="mnp")
                    for j in range(NCH):
                        nc.tensor.matmul(
                            out=mnp[:, j * 128 : (j + 1) * 128],
                            lhsT=mT[:, j * 128 : (j + 1) * 128],
                            rhs=ident[:],
                            start=(j == 0),
                            stop=(j == NCH - 1),
                            skip_group_check=True,
                        )
                    mn = actp.tile([128, EPT], bf16, tag="mnb", bufs=3, name="mn")
                    nc.vector.tensor_copy(out=mn[:], in_=mnp[:])
                    MNP[t] = mnp
                    MN[t] = mn
                t = u - 5
                if 0 <= t < T:
                    MNP.pop(t)
                    ac1 = AC1.pop(t)
                    sb = SNB[t // GMS]
                    m = t % GMS
                    agw = ps_big.tile([128, SPT + NCH + SPT], f32, tag="agw",
                                      bufs=1, name="agw")
                    for j in range(NCH):
                        nc.tensor.matmul(
                            out=agw[:, SPT + j : SPT + j + 1],
                            lhsT=ac1[:, j * 128 : (j + 1) * 128],
                            rhs=w_sb["wc2"][:],
                            start=True,
                            stop=True,
                            skip_group_check=True,
                        )
                    wcol = work.tile([128, NCH], f32, tag="wcol", name="wcol")
                    nc.vector.tensor_scalar(
                        out=wcol[:],
                        in0=agw[:, SPT : SPT + NCH],
                        scalar1=w_sb["bc2"][:],
                        scalar2=None,
                        op0=mybir.AluOpType.add,
                    )
                    dnw = work.tile([128, NCH, 3], bf16, tag="dnw", name="dnw")
                    nc.vector.tensor_tensor(
                        out=dnw[:],
                        in0=diffn[:, t * NCH : (t + 1) * NCH, :],
                        in1=wcol[:].to_broadcast([128, NCH, 3]),
                        op=mybir.AluOpType.mult,
                    )
                    mn = MN.pop(t)
                    for j in range(NCH):
                        nc.tensor.matmul(
                            out=agw[:, 0:SPT],
                            lhsT=mn[:, j * 128 : (j + 1) * 128],
                            rhs=sb[:, m, j * SPT : (j + 1) * SPT],
                            start=(j == 0),
                            stop=(j == NCH - 1),
                            skip_group_check=True,
                        )
                    for j in range(NCH):
                        nc.tensor.matmul(
                            out=agw[0:3, SPT + NCH : 2 * SPT + NCH],
                            lhsT=dnw[:, j, :],
                            rhs=sb[:, m, j * SPT : (j + 1) * SPT],
                            start=(j == 0),
                            stop=(j == NCH - 1),
                            skip_group_check=True,
                        )
                    nc.vector.tensor_copy(
                        out=mi_sb[:, t * SPT : (t + 1) * SPT], in_=agw[:, 0:SPT]
                    )
                    if t % GMS == 0:
                        CAGG8[0] = work.tile(
                            [3, GMS * SPT], f32, tag="cagg8", name="cagg8"
                        )
                    nc.vector.tensor_copy(
                        out=CAGG8[0][:, (t % GMS) * SPT : (t % GMS + 1) * SPT],
                        in_=agw[0:3, SPT + NCH : 2 * SPT + NCH],
                    )
                    if t % GMS == GMS - 1:
                        csl = work.tile([3, GMS * SPT], f32, tag="csl", name="csl")
                        g0 = (t // GMS) * GMS * SPT
                        nc.sync.dma_start(
                            out=csl[:], in_=cslotT_d[:, g0 : g0 + GMS * SPT]
                        )
                        nc.vector.tensor_tensor(
                            out=csl[:], in0=csl[:], in1=CAGG8[0][:],
                            op=mybir.AluOpType.add,
                        )
                        nc.sync.dma_start(
                            out=coutT_d[:, g0 : g0 + GMS * SPT], in_=csl[:]
                        )
            # ---- phase C: node MLP + residuals ----
            for k in range(S // EPT):
                sl = slice(k * EPT, (k + 1) * EPT)
                zn = ps_big.tile([128, EPT], f32, tag="big", name="zn")
                nc.tensor.matmul(
                    out=zn[:], lhsT=w_sb["wn1h"][:], rhs=hslotb[:, sl],
                    start=True, stop=False,
                )
                nc.tensor.matmul(
                    out=zn[:], lhsT=w_sb["wn1m"][:], rhs=mi_sb[:, sl],
                    start=False, stop=True,
                )
                an = actp.tile([128, EPT], bf16, tag="a1", bufs=3, name="an")
                nc.scalar.activation(
                    out=an[:], in_=zn[:], func=AF.Silu, bias=w_sb["bn1"][:]
                )
                zn2 = ps_big.tile([128, EPT], f32, tag="big", name="zn2")
                nc.tensor.matmul(out=zn2[:], lhsT=w_sb["wn2"][:], rhs=an[:])
                ho = work.tile([128, EPT], f32, tag="ho", name="ho")
                hres = work.tile([128, EPT], f32, tag="hs", name="hres")
                nc.sync.dma_start(out=hres[:], in_=hslotT_d[:, sl])
                nc.vector.tensor_scalar(
                    out=ho[:],
                    in0=zn2[:],
                    scalar1=w_sb["bn2"][:],
                    scalar2=None,
                    op0=mybir.AluOpType.add,
                )
                nc.vector.tensor_tensor(
                    out=ho[:], in0=ho[:], in1=hres[:], op=mybir.AluOpType.add
                )
                nc.sync.dma_start(out=houtT_d[:, sl], in_=ho[:])

    nc.compile()
    return nc


def kernel(h, coord, edge_index, We1, be1, We2, be2, Wc1, bc1, Wc2, bc2,
           Wn1, bn1, Wn2, bn2, _run=None):
    h = np.asarray(h, np.float32)
    coord = np.asarray(coord, np.float32)
    edge_index = np.asarray(edge_index)
    cores, T, S, NB, TBLR = _prep(h, coord, edge_index)
    wmap = _weights_map(We1, be1, We2, be2, Wc1, bc1, Wc2, bc2, Wn1, bn1, Wn2, bn2)

    nc = build_program(T, S, NB, TBLR)

    in_maps = []
    for cd in cores:
        m = dict(
            grp=cd["grp"].reshape(cd["grp"].shape[0], 128, -1),
            snb=cd["snb"].reshape(cd["snb"].shape[0], 128, -1),
            crd_r=cd["crd_r"].reshape(128, -1),
            crd_c=cd["crd_c"].reshape(128, -1),
            hslotT=cd["hslotT"],
            cslotT=cd["cslotT"],
        )
        for k, v in wmap.items():
            m[k] = v
        in_maps.append(m)

    if _run is None:
        from concourse.bass_utils import run_bass_kernel_spmd

        res = run_bass_kernel_spmd(nc, in_maps, list(range(NCORES)))
        outs = res.results
    else:
        outs = _run(nc, in_maps)

    N = h.shape[0]
    h_out = np.zeros((N, D), dtype=np.float32)
    coord_out = np.zeros((N, 3), dtype=np.float32)
    for cd, om in zip(cores, outs):
        sn_ = cd["slot_node"]
        v = sn_ >= 0
        h_out[sn_[v]] = np.asarray(om["houtT"]).T[v]
        coord_out[sn_[v]] = np.asarray(om["coutT"]).T[v]
    return h_out, coord_out


# revision 28
# speedup vs baseline: 1.8412x; 1.0260x over previous
"""EGCL (E(n)-equivariant graph conv layer) Trainium2 Bass kernel.

Strategy (edge-parallel, sharded by destination node r):
  - Host: sort edges by r, split nodes into 8 contiguous ranges with ~equal
    edge counts. Each core owns all edges pointing INTO its node range, so
    every aggregation (m_i segment-sum, coord update) is core-local: no
    collectives at all.
  - Edges are packed into 512-edge "macro tiles" aligned to node boundaries,
    each covering <=64 node slots. One-hot segment matrices (S / S^T) are
    pure indexing data and are built on the host and streamed in.
  - Device pipeline per macro tile (transposed activations: features on
    partitions, edges on the free dim):
      z1^T = P1[r]-expand (S^T matmul) + We1_c^T @ h_c^T (transposed gather)
             + rad outer-product (via PE-transposed rad slabs + masked mms)
      a1^T = silu(z1^T + be1)         [ACT, bias fused]
      m^T  = silu(We2^T @ a1^T + be2)
      w    = coord-MLP from m^T, ending in natural-layout w via per-chunk mms
      m_ij natural via identity-matmul transposes -> segment-matmul agg
  - Node MLP over slot columns, residuals added in f32.
Host does indexing/permutation/casting only - all FLOPs are on device.
"""

import sys

sys.path.insert(0, "/opt/trn_rl_repo")

import numpy as np
import ml_dtypes

D = 128
EPT = 512  # edges per macro tile
SPT = 64  # node slots per macro tile
NCH = EPT // 128  # 128-edge chunks per macro tile
GMS = 4  # macro tiles per packed stream group
NCORES = 8
EPS = 1e-8

BF16 = ml_dtypes.bfloat16


def _pack_core(nlo, nhi, counts, cum):
    """Greedy-pack nodes [nlo, nhi) into macro tiles of <=EPT edges and
    <=SPT consecutive nodes. Returns list of (node_start, node_end)."""
    tiles = []
    a = nlo
    while a < nhi:
        b = a
        edges = 0
        while b < nhi and (b - a) < SPT and edges + counts[b] <= EPT:
            edges += counts[b]
            b += 1
        assert b > a, f"node {a} has degree {counts[a]} > {EPT}"
        tiles.append((a, b))
        a = b
    return tiles


def _prep(h, coord, edge_index):
    """Host-side sharding / index prep. Pure permutation+casting, no math."""
    N = h.shape[0]
    E = edge_index.shape[1]
    r = edge_index[0].astype(np.int64)
    c = edge_index[1].astype(np.int64)

    perm = np.argsort(r, kind="stable")
    r_s = r[perm]
    c_s = c[perm]
    counts = np.bincount(r_s, minlength=N)
    cum = np.concatenate([[0], np.cumsum(counts)])  # cum[n] = first edge of n

    # split nodes into NCORES ranges with ~equal edge counts
    bounds = [0]
    for k in range(1, NCORES):
        t = E * k // NCORES
        n = int(np.searchsorted(cum, t, side="left"))
        bounds.append(min(max(n, bounds[-1]), N))
    bounds.append(N)

    core_tiles = [
        _pack_core(bounds[k], bounds[k + 1], counts, cum) for k in range(NCORES)
    ]
    T = max(len(t) for t in core_tiles)
    T = ((T + 7) // 8) * 8  # multiple of 8: stream groups and EPT-wide slots
    S = T * SPT
    NB = 0

    h_bf = h.astype(BF16)
    cores = []
    for k in range(NCORES):
        tiles = core_tiles[k]
        lid = np.full((T, EPT), 127, dtype=np.int64)  # local slot id, 127=pad
        cidx = np.zeros((T, EPT), dtype=np.int64)  # c node id (pad -> 0)
        slot_node = np.full(S, -1, dtype=np.int64)
        for t, (a, b) in enumerate(tiles):
            e0, e1 = cum[a], cum[b]
            ne = e1 - e0
            lid[t, :ne] = r_s[e0:e1] - a
            cidx[t, :ne] = c_s[e0:e1]
            slot_node[t * SPT : t * SPT + (b - a)] = np.arange(a, b)

        # one-hot segment matrices
        sl = np.arange(SPT)
        st = (lid[:, None, :] == sl[None, :, None]).astype(BF16)  # [T,64,512]
        lid_r = lid.reshape(T, NCH, 128)
        sn = (lid_r[:, :, :, None] == sl[None, None, None, :]).astype(
            BF16
        )  # [T,NCH,128,64]
        sn = sn.transpose(0, 2, 1, 3).reshape(T, 128, NCH * SPT)  # [T,128,NCH*64]

        # host-expanded h_c in transposed layout: col t*EPT+e = h_bf[c(t, e)]
        hct = h_bf[cidx].reshape(T * EPT, D).T  # [128, T*EPT]
        # packed per-group stream: hcT + ST (parity-placed); SN separate
        G = T // GMS
        grp = np.zeros((G, 128, GMS, 2 * EPT), dtype=BF16)
        snb = np.zeros((G, 128, GMS, NCH * SPT), dtype=BF16)
        hct3 = hct.reshape(128, T, EPT)
        for t in range(T):
            gg, m = divmod(t, GMS)
            grp[gg, :, m, 0:EPT] = hct3[:, t, :]
            pb = SPT * (t % 2)
            grp[gg, pb : pb + SPT, m, EPT : 2 * EPT] = st[t]
            snb[gg, :, m, :] = sn[t]

        # host-expanded per-edge coords, layout [128, T*NCH, 3]
        rn = slot_node[(np.arange(T)[:, None] * SPT + lid).clip(0, S - 1)]
        rn = np.where(lid < SPT, rn, 0).clip(0, N - 1)  # pad -> node 0
        crd_r = coord[rn]  # [T,EPT,3]
        crd_r[lid >= SPT] = 0.0
        crd_c = coord[cidx]  # [T,EPT,3]
        cr_dev = crd_r.reshape(T * NCH, 128, 3).transpose(1, 0, 2)  # [128,T4,3]
        cc_dev = crd_c.reshape(T * NCH, 128, 3).transpose(1, 0, 2)

        sn_valid = slot_node >= 0
        hslot = np.zeros((S, D), dtype=np.float32)
        hslot[sn_valid] = h[slot_node[sn_valid]]
        cslot = np.zeros((S, 3), dtype=np.float32)
        cslot[sn_valid] = coord[slot_node[sn_valid]]

        cores.append(
            dict(
                lid=lid,
                cidx=cidx,
                slot_node=slot_node,
                grp=np.ascontiguousarray(grp),
                snb=np.ascontiguousarray(snb),
                crd_r=np.ascontiguousarray(cr_dev.astype(np.float32)),
                crd_c=np.ascontiguousarray(cc_dev.astype(np.float32)),
                hslotT=np.ascontiguousarray(hslot.T),  # [128,S] f32
                cslotT=np.ascontiguousarray(cslot.T),  # [3,S] f32
            )
        )

    return cores, T, S, NB, 0


def _weights_map(We1, be1, We2, be2, Wc1, bc1, Wc2, bc2, Wn1, bn1, Wn2, bn2):
    """Per-core replicated weight tensors (bf16 mats, f32 bias columns)."""
    col = lambda v: np.ascontiguousarray(
        np.broadcast_to(np.asarray(v, np.float32).reshape(-1, 1), (D, 1))
        if np.asarray(v).size in (1, D)
        else v
    )
    zcat = np.zeros((32, 32, D), dtype=BF16)
    for g in range(32):
        zcat[g, g] = We1[2 * D].astype(BF16)
    zcat = zcat.transpose(1, 0, 2).reshape(32, 32 * D)  # [k, g*D]
    return dict(
        we1r=We1[:D].astype(BF16),
        we1c=We1[D : 2 * D].astype(BF16),
        zcat=zcat,
        we2=We2.astype(BF16),
        wc1=Wc1.astype(BF16),
        wc2=Wc2.astype(BF16),
        wn1h=Wn1[:D].astype(BF16),
        wn1m=Wn1[D : 2 * D].astype(BF16),
        wn2=Wn2.astype(BF16),
        be1=col(be1),
        be2=col(be2),
        bc1=col(bc1),
        bc2=col(np.full(D, float(np.asarray(bc2).reshape(-1)[0]), np.float32)),
        bn1=col(bn1),
        bn2=col(bn2),
    )


def build_program(T, S, NB, TBLR):
    import concourse.bass as bass
    import concourse.mybir as mybir
    import concourse.tile as tile
    from concourse import bacc
    from concourse.masks import make_identity

    f32 = mybir.dt.float32
    bf16 = mybir.dt.bfloat16
    AF = mybir.ActivationFunctionType
    T4 = T * NCH
    NSLAB = (T4 + 31) // 32

    nc = bacc.Bacc("TRN2", target_bir_lowering=False, debug=False)
    g = lambda n, s, d: nc.declare_dram_parameter(n, list(s), d, isOutput=False)
    G = T // GMS
    grp_d = g("grp", (G, 128, GMS * 2 * EPT), bf16)
    snb_d = g("snb", (G, 128, GMS * NCH * SPT), bf16)
    crd_r_d = g("crd_r", (128, T4 * 3), f32)
    crd_c_d = g("crd_c", (128, T4 * 3), f32)
    hslotT_d = g("hslotT", (128, S), f32)
    cslotT_d = g("cslotT", (3, S), f32)
    wnames = [
        ("we1r", (D, D), bf16),
        ("we1c", (D, D), bf16),
        ("zcat", (32, 32 * D), bf16),
        ("we2", (D, D), bf16),
        ("wc1", (D, D), bf16),
        ("wc2", (D, 1), bf16),
        ("wn1h", (D, D), bf16),
        ("wn1m", (D, D), bf16),
        ("wn2", (D, D), bf16),
        ("be1", (D, 1), f32),
        ("be2", (D, 1), f32),
        ("bc1", (D, 1), f32),
        ("bc2", (D, 1), f32),
        ("bn1", (D, 1), f32),
        ("bn2", (D, 1), f32),
    ]
    wd = {n: g(n, s, d) for n, s, d in wnames}
    houtT_d = nc.declare_dram_parameter("houtT", [128, S], f32, isOutput=True)
    coutT_d = nc.declare_dram_parameter("coutT", [3, S], f32, isOutput=True)

    with tile.TileContext(nc) as tc:
        with (
            tc.tile_pool(name="res", bufs=1) as res,
            tc.tile_pool(name="work", bufs=2) as work,
            tc.tile_pool(name="gath", bufs=3) as gath,
            tc.tile_pool(name="act", bufs=2) as actp,
            tc.tile_pool(name="ps_big", bufs=5, space="PSUM") as ps_big,
        ):
            # ---- phase 0: residents ----
            w_sb = {}
            for n, s, d in wnames:
                w_sb[n] = res.tile(list(s), d, name=f"w_{n}")
                nc.sync.dma_start(out=w_sb[n][:], in_=wd[n][:])
            ident = res.tile([128, 128], bf16, name="ident")
            make_identity(nc, ident[:])
            identf = res.tile([128, 128], f32, name="identf")
            make_identity(nc, identf[:])

            hslotb = res.tile([128, S], bf16, name="hslotb")
            for k in range(S // EPT):
                hs_t = work.tile([128, EPT], f32, tag="hs", name="hs_t")
                nc.sync.dma_start(
                    out=hs_t[:], in_=hslotT_d[:, k * EPT : (k + 1) * EPT]
                )
                nc.vector.tensor_copy(
                    out=hslotb[:, k * EPT : (k + 1) * EPT], in_=hs_t[:]
                )
            mi_sb = res.tile([128, S], bf16, name="mi_sb")
            diffn = res.tile([128, T4, 3], bf16, name="diffn")
            radt = res.tile([32, NSLAB * 128], bf16, name="radt")
            p1 = res.tile([128, (T + 1) // 2, D], bf16, name="p1")

            # ---- P1 = h_slot @ We1_r (two macro tiles per 128-slot chunk) ----
            for k in range(S // 128):
                pk = ps_big.tile([128, 512], f32, tag="big", name="pk")
                nc.tensor.matmul(
                    out=pk[:, 0:128],
                    lhsT=hslotb[:, k * 128 : (k + 1) * 128],
                    rhs=w_sb["we1r"][:],
                    start=True,
                    stop=True,
                )
                nc.vector.tensor_copy(out=p1[:, k, :], in_=pk[:, 0:128])

            # ---- phase A: coord diff / rad / normalized diff ----
            crd_r = work.tile([128, T4, 3], f32, tag="crd", name="crd_r")
            crd_c = work.tile([128, T4, 3], f32, tag="crd", name="crd_c")
            nc.sync.dma_start(out=crd_r[:], in_=crd_r_d[:])
            nc.sync.dma_start(out=crd_c[:], in_=crd_c_d[:])
            rad = work.tile([128, T4], f32, tag="rad", name="rad")
            diff = work.tile([128, T4, 3], f32, tag="diff", name="diff")
            nc.vector.tensor_tensor(
                out=diff[:], in0=crd_r[:], in1=crd_c[:], op=mybir.AluOpType.subtract
            )
            sq = work.tile([128, T4, 3], f32, tag="crd", name="sq")
            nc.vector.tensor_tensor(
                out=sq[:], in0=diff[:], in1=diff[:], op=mybir.AluOpType.mult
            )
            nc.vector.reduce_sum(out=rad[:], in_=sq[:], axis=mybir.AxisListType.X)
            rnorm = work.tile([128, T4], f32, tag="rn", name="rnorm")
            nc.scalar.activation(out=rnorm[:], in_=rad[:], func=AF.Sqrt)
            nc.vector.tensor_scalar_add(out=rnorm[:], in0=rnorm[:], scalar1=EPS)
            nc.vector.reciprocal(out=rnorm[:], in_=rnorm[:])
            nc.vector.tensor_tensor(
                out=diffn[:],
                in0=diff[:],
                in1=rnorm[:].to_broadcast([128, T4, 3]),
                op=mybir.AluOpType.mult,
            )
            # rad slabs: radt[k, s*128 + m] = rad[m, 32s + k]
            for s in range(NSLAB):
                w32 = min(32, T4 - s * 32)
                rtp = ps_big.tile([32, 512], f32, tag="big", name="rtp")
                nc.tensor.transpose(
                    out=rtp[:w32, 0:128],
                    in_=rad[:, s * 32 : s * 32 + w32],
                    identity=identf[:],
                )
                nc.vector.tensor_copy(
                    out=radt[:w32, s * 128 : (s + 1) * 128], in_=rtp[:w32, 0:128]
                )

            # ---- phase B: software-pipelined macro tiles ----
            # stage offsets (iteration u runs t at lag 0):
            #   u:   z1-MMs(t)
            #   u+1: a1-silu(t)
            #   u+2: z2-MM(t), mT-silu(t)
            #   u+3: c1-MM(t), ac1-silu(t)
            #   u+4: mn-MMs(t), mn-cast(t)
            #   u+5: w-MMs(t), wcol/dnw(t), mi-MMs(t), cagg-MMs(t), copies(t)
            LAG = 5
            GRP, SNB, Z1, A1, Z2, MT, C1, AC1, MNP, MN, DNW = ({} for _ in range(11))
            CAGG8 = {}

            def load_group(gg):
                if gg >= G:
                    return
                gt = gath.tile([128, GMS, 2 * EPT], bf16, tag="grp", name="gt")
                nc.sync.dma_start(out=gt[:], in_=grp_d[gg])
                GRP[gg] = gt
                sb = gath.tile(
                    [128, GMS, NCH * SPT], bf16, tag="snb", bufs=4, name="sb"
                )
                nc.sync.dma_start(out=sb[:], in_=snb_d[gg])
                SNB[gg] = sb

            load_group(0)
            for u in range(T + LAG):
                t = u
                if t < T:
                    if t % GMS == 0:
                        load_group(t // GMS + 1)
                    gt = GRP[t // GMS]
                    m = t % GMS
                    pb = SPT * (t % 2)
                    z1 = ps_big.tile([128, EPT], f32, tag="big", name="z1")
                    nc.tensor.matmul(
                        out=z1[:],
                        lhsT=p1[pb : pb + SPT, t // 2, :],
                        rhs=gt[pb : pb + SPT, m, EPT : 2 * EPT],
                        start=True,
                        stop=False,
                    )
                    for j in range(NCH):
                        gch = t * NCH + j
                        nc.tensor.matmul(
                            out=z1[:, j * 128 : (j + 1) * 128],
                            lhsT=w_sb["zcat"][:, (gch % 32) * D : (gch % 32 + 1) * D],
                            rhs=radt[:, (gch // 32) * 128 : (gch // 32 + 1) * 128],
                            start=False,
                            stop=False,
                            skip_group_check=True,
                        )
                    nc.tensor.matmul(
                        out=z1[:],
                        lhsT=w_sb["we1c"][:],
                        rhs=gt[:, m, 0:EPT],
                        start=False,
                        stop=True,
                    )
                    Z1[t] = z1
                t = u - 1
                if 0 <= t < T:
                    a1 = actp.tile([128, EPT], bf16, tag="a1", bufs=3, name="a1")
                    nc.scalar.activation(
                        out=a1[:], in_=Z1.pop(t)[:], func=AF.Silu, bias=w_sb["be1"][:]
                    )
                    A1[t] = a1
                t = u - 2
                if 0 <= t < T:
                    z2 = ps_big.tile([128, EPT], f32, tag="big", name="z2")
                    nc.tensor.matmul(out=z2[:], lhsT=w_sb["we2"][:], rhs=A1.pop(t)[:])
                    mT = actp.tile([128, EPT], bf16, tag="mT", bufs=4, name="mT")
                    nc.scalar.activation(
                        out=mT[:], in_=z2[:], func=AF.Silu, bias=w_sb["be2"][:]
                    )
                    Z2[t] = z2
                    MT[t] = mT
                t = u - 3
                if 0 <= t < T:
                    Z2.pop(t)
                    c1 = ps_big.tile([128, EPT], f32, tag="big", name="c1")
                    nc.tensor.matmul(out=c1[:], lhsT=w_sb["wc1"][:], rhs=MT[t][:])
                    ac1 = actp.tile([128, EPT], bf16, tag="ac1", bufs=4, name="ac1")
                    nc.scalar.activation(
                        out=ac1[:], in_=c1[:], func=AF.Silu, bias=w_sb["bc1"][:]
                    )
                    C1[t] = c1
                    AC1[t] = ac1
                t = u - 4
                if 0 <= t < T:
                    C1.pop(t)
                    mT = MT.pop(t)
                    mnp = ps_big.tile([128, EPT], f32, tag="mn", bufs=2, name="mnp")
                    for j in range(NCH):
                        nc.tensor.matmul(
                            out=mnp[:, j * 128 : (j + 1) * 128],
                            lhsT=mT[:, j * 128 : (j + 1) * 128],
                            rhs=ident[:],
                            start=(j == 0),
                            stop=(j == NCH - 1),
                            skip_group_check=True,
                        )
                    mn = actp.tile([128, EPT], bf16, tag="mnb", bufs=3, name="mn")
                    nc.vector.tensor_copy(out=mn[:], in_=mnp[:])
                    MNP[t] = mnp
                    MN[t] = mn
                t = u - 5
                if 0 <= t < T:
                    MNP.pop(t)
                    ac1 = AC1.pop(t)
                    sb = SNB[t // GMS]
                    m = t % GMS
                    agw = ps_big.tile([128, SPT + NCH + SPT], f32, tag="agw",
                                      bufs=1, name="agw")
                    for j in range(NCH):
                        nc.tensor.matmul(
                            out=agw[:, SPT + j : SPT + j + 1],
                            lhsT=ac1[:, j * 128 : (j + 1) * 128],
                            rhs=w_sb["wc2"][:],
                            start=True,
                            stop=True,
                            skip_group_check=True,
                        )
                    wcol = work.tile([128, NCH], f32, tag="wcol", name="wcol")
                    nc.vector.tensor_scalar(
                        out=wcol[:],
                        in0=agw[:, SPT : SPT + NCH],
                        scalar1=w_sb["bc2"][:],
                        scalar2=None,
                        op0=mybir.AluOpType.add,
                    )
                    dnw = work.tile([128, NCH, 3], bf16, tag="dnw", name="dnw")
                    nc.vector.tensor_tensor(
                        out=dnw[:],
                        in0=diffn[:, t * NCH : (t + 1) * NCH, :],
                        in1=wcol[:].to_broadcast([128, NCH, 3]),
                        op=mybir.AluOpType.mult,
                    )
                    mn = MN.pop(t)
                    for j in range(NCH):
                        nc.tensor.matmul(
                            out=agw[:, 0:SPT],
                            lhsT=mn[:, j * 128 : (j + 1) * 128],
                            rhs=sb[:, m, j * SPT : (j + 1) * SPT],
                            start=(j == 0),
                            stop=(j == NCH - 1),
                            skip_group_check=True,
                        )
                    for j in range(NCH):
                        nc.tensor.matmul(
                            out=agw[0:3, SPT + NCH : 2 * SPT + NCH],
                            lhsT=dnw[:, j, :],
                            rhs=sb[:, m, j * SPT : (j + 1) * SPT],
                            start=(j == 0),
                            stop=(j == NCH - 1),
                            skip_group_check=True,
                        )
                    nc.vector.tensor_copy(
                        out=mi_sb[:, t * SPT : (t + 1) * SPT], in_=agw[:, 0:SPT]
                    )
                    if t % GMS == 0:
                        CAGG8[0] = work.tile(
                            [3, GMS * SPT], f32, tag="cagg8", name="cagg8"
                        )
                    nc.vector.tensor_copy(
                        out=CAGG8[0][:, (t % GMS) * SPT : (t % GMS + 1) * SPT],
                        in_=agw[0:3, SPT + NCH : 2 * SPT + NCH],
                    )
                    if t % GMS == GMS - 1:
                        csl = work.tile([3, GMS * SPT], f32, tag="csl", name="csl")
                        g0 = (t // GMS) * GMS * SPT
                        nc.sync.dma_start(
                            out=csl[:], in_=cslotT_d[:, g0 : g0 + GMS * SPT]
                        )
                        nc.vector.tensor_tensor(
                            out=csl[:], in0=csl[:], in1=CAGG8[0][:],
                            op=mybir.AluOpType.add,
                        )
                        nc.sync.dma_start(
                            out=coutT_d[:, g0 : g0 + GMS * SPT], in_=csl[:]
                        )
            # ---- phase C: node MLP + residuals ----
            for k in range(S // EPT):
                sl = slice(k * EPT, (k + 1) * EPT)
                zn = ps_big.tile([128, EPT], f32, tag="big", name="zn")
                nc.tensor.matmul(
                    out=zn[:], lhsT=w_sb["wn1h"][:], rhs=hslotb[:, sl],
                    start=True, stop=False,
                )
                nc.tensor.matmul(
                    out=zn[:], lhsT=w_sb["wn1m"][:], rhs=mi_sb[:, sl],
                    start=False, stop=True,
                )
                an = actp.tile([128, EPT], bf16, tag="a1", bufs=3, name="an")
                nc.scalar.activation(
                    out=an[:], in_=zn[:], func=AF.Silu, bias=w_sb["bn1"][:]
                )
                zn2 = ps_big.tile([128, EPT], f32, tag="big", name="zn2")
                nc.tensor.matmul(out=zn2[:], lhsT=w_sb["wn2"][:], rhs=an[:])
                ho = work.tile([128, EPT], f32, tag="ho", name="ho")
                hres = work.tile([128, EPT], f32, tag="hs", name="hres")
                nc.sync.dma_start(out=hres[:], in_=hslotT_d[:, sl])
                nc.vector.tensor_scalar(
                    out=ho[:],
                    in0=zn2[:],
                    scalar1=w_sb["bn2"][:],
                    scalar2=None,
                    op0=mybir.AluOpType.add,
                )
                nc.vector.tensor_tensor(
                    out=ho[:], in0=ho[:], in1=hres[:], op=mybir.AluOpType.add
                )
                nc.sync.dma_start(out=houtT_d[:, sl], in_=ho[:])

    nc.compile()
    return nc


def kernel(h, coord, edge_index, We1, be1, We2, be2, Wc1, bc1, Wc2, bc2,
           Wn1, bn1, Wn2, bn2, _run=None):
    h = np.asarray(h, np.float32)
    coord = np.asarray(coord, np.float32)
    edge_index = np.asarray(edge_index)
    cores, T, S, NB, TBLR = _prep(h, coord, edge_index)
    wmap = _weights_map(We1, be1, We2, be2, Wc1, bc1, Wc2, bc2, Wn1, bn1, Wn2, bn2)

    nc = build_program(T, S, NB, TBLR)

    in_maps = []
    for cd in cores:
        m = dict(
            grp=cd["grp"].reshape(cd["grp"].shape[0], 128, -1),
            snb=cd["snb"].reshape(cd["snb"].shape[0], 128, -1),
            crd_r=cd["crd_r"].reshape(128, -1),
            crd_c=cd["crd_c"].reshape(128, -1),
            hslotT=cd["hslotT"],
            cslotT=cd["cslotT"],
        )
        for k, v in wmap.items():
            m[k] = v
        in_maps.append(m)

    if _run is None:
        from concourse.bass_utils import run_bass_kernel_spmd

        res = run_bass_kernel_spmd(nc, in_maps, list(range(NCORES)))
        outs = res.results
    else:
        outs = _run(nc, in_maps)

    N = h.shape[0]
    h_out = np.zeros((N, D), dtype=np.float32)
    coord_out = np.zeros((N, 3), dtype=np.float32)
    for cd, om in zip(cores, outs):
        sn_ = cd["slot_node"]
        v = sn_ >= 0
        h_out[sn_[v]] = np.asarray(om["houtT"]).T[v]
        coord_out[sn_[v]] = np.asarray(om["coutT"]).T[v]
    return h_out, coord_out
